# revision 1
# baseline (speedup 1.0000x reference)
"""Trainium2 Bass kernel for nn_EnhancedAdaptiveGate.

Reference computation (per sample b of 64, channels C=128, length L=4096):
  stats = concat([mean, std, skew, diff_std, recent_mean, recent_std])  # [B, 768]
  alpha = sigmoid(gelu(gelu(stats @ W1 + b1) @ W2 + b2) @ W3 + b3)      # [B, 128]

Sharding: data-parallel over batch — 8 samples per NeuronCore, MLP weights
replicated, no cross-core communication. Each core computes 8 output rows;
the host concatenates.

Per-core algorithm (folded-contiguous layout):
  - x[s] loaded as bf16 via cast-DMA with partition p holding L-rows
    [32p, 32p+32) — so the lag product x_t*x_{t+1} is a free-dim shift
    (no partition-shift copies), and the "recent" window (t >= 3072) is
    exactly partitions 96..127.
  - ACT: x2 = square(x); DVE: x3 = x2*x, prod = x*shift(x) (all bf16).
  - PE: a single stationary [128,2] masked-ones matrix reduces every
    variant per chunk: column 0 sums partitions 0..95 (t < 3072),
    column 1 sums partitions 96..127 (recent). PSUM accumulates the 32
    chunks -> per-channel sums S1, S2, S3, P split into (a=non-recent,
    r=recent) parts.
  - diff-std via the telescoping identity D2 = 2*S2 - x0^2 - xL^2 - 2*P.
  - Small fp32 epilogue computes the 6 stats batched across samples
    (samples live on partitions), transposes them with the PE, and runs
    the tiny MLP on the PE (gelu via erf so one ACT table set covers
    erf+sigmoid; sqrt comes from the sqrt set used by the main loop).
"""

import numpy as np

import concourse.bass as bass
import concourse.bacc as bacc
import concourse.tile as tile
from concourse import mybir
from concourse.bass_utils import run_bass_kernel_spmd

F32 = mybir.dt.float32
BF16 = mybir.dt.bfloat16
FP8 = mybir.dt.float8e4
ALU = mybir.AluOpType
ACT = mybir.ActivationFunctionType

B, L, C = 64, 4096, 128
NCORES = 8
BS = B // NCORES            # samples per core
G = 32                      # L-rows per partition (folded layout)
EPS = 1e-8

N = float(L)                # 4096
NR = float(L // 4)          # 1024
ND = float(L - 1)           # 4095


SUB = 2                     # sub-tiles per sample (pipeline granularity)
GS = G // SUB               # g-blocks per sub-tile


def _sample_body(nc, big, psp, s, x, ones2, ones2f8, bnd_pool):
    """Sub-tiled main loop for one sample; returns the [2, 4C] PSUM
    accumulator (row 0: t<3072 part, row 1: recent part; free blocks
    S1|S2|S3|P). x and x^2 stream as bf16; x^3 and the lag products
    stream as fp8e4m3 DoubleRow matmuls (two g-blocks per matmul)."""
    psA = psp.tile([2, 4 * C], F32, tag="psA")
    xr = x[s].rearrange("(p g) c -> p g c", g=G)
    first = True
    tiles = []
    for k in range(SUB):
        xb = big.tile([128, GS, C], BF16, tag=f"xb{k % 2}")
        nc.gpsimd.dma_start(out=xb[:], in_=xr[:, k * GS:(k + 1) * GS, :])
        x2b = big.tile([128, GS, C], BF16, tag=f"x2b{k % 2}")
        nc.scalar.activation(
            out=x2b.rearrange("p g c -> p (g c)"),
            in_=xb.rearrange("p g c -> p (g c)"),
            func=ACT.Square,
        )
        x3b = big.tile([128, GS, C], BF16, tag=f"x3b{k % 2}")
        nc.vector.tensor_mul(
            out=x3b.rearrange("p g c -> p (g c)"),
            in0=x2b.rearrange("p g c -> p (g c)"),
            in1=xb.rearrange("p g c -> p (g c)"),
        )
        prodb = big.tile([128, GS, C], BF16, tag=f"prodb{k % 2}")
        nc.vector.tensor_mul(
            out=prodb[:, 0:GS - 1, :].rearrange("p g c -> p (g c)"),
            in0=xb[:, 0:GS - 1, :].rearrange("p g c -> p (g c)"),
            in1=xb[:, 1:GS, :].rearrange("p g c -> p (g c)"),
        )
        tiles.append((xb, x2b, x3b, prodb))
    xb0 = tiles[0][0]
    xb1 = tiles[1][0]
    # cross-sub-tile lag pairs fill sub-0's last prod slot
    nc.vector.tensor_mul(tiles[0][3][:, GS - 1, :], xb0[:, GS - 1, :], xb1[:, 0, :])
    # partition-boundary pairs x[32p+31]*x[32(p+1)] fill sub-1's last slot
    # (pair t=32p+31 has the same recent-mask as partition p, so it may live
    # on partition p); the nonexistent t=L-1 pair on partition 127 is zeroed.
    bnd = bnd_pool.tile([127, C], BF16, tag="bnd")
    nc.sync.dma_start(out=bnd[:], in_=xb0[1:128, 0, :])
    nc.vector.memset(tiles[1][3][96:128, GS - 1, :], 0.0)
    nc.vector.tensor_mul(tiles[1][3][0:127, GS - 1, :],
                         xb1[0:127, GS - 1, :], bnd[:])

    for k in range(SUB):
        xb, x2b, x3b, prodb = tiles[k]
        lastk = k == SUB - 1
        for g in range(GS):
            last = lastk and g == GS - 1
            nc.tensor.matmul(psA[0:2, 0:C], ones2, xb[:, g, :],
                             start=first, stop=last, skip_group_check=True)
            first = False
            nc.tensor.matmul(psA[0:2, C:2 * C], ones2, x2b[:, g, :],
                             start=False, stop=last, skip_group_check=True)
        for g in range(GS):
            last = lastk and g == GS - 1
            nc.tensor.matmul(psA[0:2, 2 * C:3 * C], ones2, x3b[:, g, :],
                             start=False, stop=last, skip_group_check=True)
            nc.tensor.matmul(psA[0:2, 3 * C:4 * C], ones2, prodb[:, g, :],
                             start=False, stop=last, skip_group_check=True)
    return psA


def build():
    nc = bacc.Bacc("TRN2", target_bir_lowering=False, debug=False)
    x = nc.declare_dram_parameter("x", [BS, L, C], F32, isOutput=False)
    W1 = nc.declare_dram_parameter("W1", [6 * C, 128], F32, isOutput=False)
    b1 = nc.declare_dram_parameter("b1", [128], F32, isOutput=False)
    W2 = nc.declare_dram_parameter("W2", [128, 32], F32, isOutput=False)
    b2 = nc.declare_dram_parameter("b2", [32], F32, isOutput=False)
    W3 = nc.declare_dram_parameter("W3", [32, C], F32, isOutput=False)
    b3 = nc.declare_dram_parameter("b3", [C], F32, isOutput=False)
    out = nc.declare_dram_parameter("out", [C, BS], F32, isOutput=True)

    eye8 = nc.inline_tensor(np.eye(8, dtype=np.float32), name="eye8")

    with tile.TileContext(nc) as tc:
        with (
            tc.tile_pool(name="big", bufs=4) as big,
            tc.tile_pool(name="bndp", bufs=4) as bndp,
            tc.tile_pool(name="psum", bufs=4, space="PSUM") as psp,
            tc.tile_pool(name="stage", bufs=4) as stage,
            tc.tile_pool(name="fin", bufs=1) as fin,
            tc.tile_pool(name="pse", bufs=1, space="PSUM") as pse,
        ):
            ones2 = fin.tile([128, 2], BF16, tag="ones2")
            nc.vector.memset(ones2[:], 0.0)
            nc.vector.memset(ones2[0:96, 0:1], 1.0)
            nc.vector.memset(ones2[96:128, 1:2], 1.0)
            # fp8 masked-ones for DoubleRow (j-dim stride padded to 16B)
            ones2f8 = fin.tile([128, 2, 16], FP8, tag="ones2f8")
            nc.vector.memset(ones2f8[:], 0.0)
            nc.vector.memset(ones2f8[0:96, :, 0:1], 1.0)
            nc.vector.memset(ones2f8[96:128, :, 1:2], 1.0)
            # trigger the ACT table load before the first data arrives
            warm = fin.tile([1, 8], F32, tag="warm")
            nc.vector.memset(warm[:], 0.5)
            nc.scalar.activation(out=warm[:], in_=warm[:], func=ACT.Square)
            ones1 = fin.tile([1, 8], F32, tag="ones1")
            nc.vector.memset(ones1, 1.0)
            idsb = fin.tile([8, 8], F32, tag="idsb")
            nc.sync.dma_start(out=idsb[:], in_=eye8[:])

            # MLP weights on SBUF
            w1sb = fin.tile([128, 6, 128], F32, tag="w1sb")
            nc.sync.dma_start(out=w1sb[:], in_=W1.rearrange("(k p) j -> p k j", p=128))
            w2sb = fin.tile([128, 32], F32, tag="w2sb")
            nc.sync.dma_start(out=w2sb[:], in_=W2[:])
            w3sb = fin.tile([32, C], F32, tag="w3sb")
            nc.sync.dma_start(out=w3sb[:], in_=W3[:])
            b1sb = fin.tile([1, 128], F32, tag="b1sb")
            nc.sync.dma_start(out=b1sb[:], in_=b1.rearrange("(a c) -> a c", a=1))
            b2sb = fin.tile([1, 32], F32, tag="b2sb")
            nc.sync.dma_start(out=b2sb[:], in_=b2.rearrange("(a c) -> a c", a=1))
            b3sb = fin.tile([1, C], F32, tag="b3sb")
            nc.sync.dma_start(out=b3sb[:], in_=b3.rearrange("(a c) -> a c", a=1))

            # Per-sample sums live on partitions 0-7 (sample s), with one
            # 128-wide free block per quantity (engines require all operands
            # of an op at the same base partition, so stacking is by free dim):
            #   raw2 blocks: S1a|S2a|S3a|Pa|S1r|S2r|S3r|Pr
            raw2 = fin.tile([8, 8 * C], F32, tag="raw2")
            xr = fin.tile([8, 2 * C], F32, tag="xr")    # x0 | xL
            nc.sync.dma_start(out=xr[:, 0:C], in_=x[:, 0, :])
            nc.sync.dma_start(out=xr[:, C:2 * C], in_=x[:, L - 1, :])

            # per-block constants (broadcast tiles) for the batched stats math
            K1 = fin.tile([8, 3 * C], F32, tag="K1")
            KNEG = fin.tile([8, 3 * C], F32, tag="KNEG")
            KINV = fin.tile([8, 3 * C], F32, tag="KINV")
            KEPS = fin.tile([8, 3 * C], F32, tag="KEPS")
            for t, vals in (
                (K1, (1.0 / N, 1.0 / NR, 1.0 / ND)),
                (KNEG, (-N, -NR, -ND)),
                (KINV, (1.0 / (N - 1), 1.0 / (NR - 1), 1.0 / (ND - 1))),
                (KEPS, (EPS, EPS, 0.0)),
            ):
                for i, v in enumerate(vals):
                    nc.vector.memset(t[:, C * i:C * (i + 1)], v)

            # ---------------- main loop over samples ----------------
            for s in range(BS):
                psA = _sample_body(nc, big, psp, s, x, ones2, ones2f8, bndp)
                stA = stage.tile([2, 4 * C], F32, tag="stA")
                nc.scalar.copy(stA[:], psA[:])
                # raw2 blocks: S1a|S2a|S3a|Pa | S1r|S2r|S3r|Pr
                for r in range(2):
                    nc.sync.dma_start(
                        out=raw2[s:s + 1, 4 * C * r:4 * C * (r + 1)],
                        in_=stA[r:r + 1, :],
                    )

            # ---------------- stats epilogue (free-stacked blocks) ------------
            FU = fin.tile([8, 4 * C], F32, tag="FU")     # S1|S2|S3|P full sums
            WA = fin.tile([8, 3 * C], F32, tag="WA")     # S1|S1r|xL-x0
            WB = fin.tile([8, 3 * C], F32, tag="WB")     # S2|S2r|D2
            MU = fin.tile([8, 3 * C], F32, tag="MU")     # mean|rmean|mean_d
            SQ = fin.tile([8, 2 * C], F32, tag="SQ")     # x0^2|xL^2
            SQ2 = fin.tile([8, 3 * C], F32, tag="SQ2")
            V = fin.tile([8, 3 * C], F32, tag="V")
            STD = fin.tile([8, 3 * C], F32, tag="STD")   # std|rstd|diff_std
            TMP = fin.tile([8, 3 * C], F32, tag="TMP")
            SKW = fin.tile([8, C], F32, tag="SKW")

            nc.vector.tensor_add(FU[:], raw2[:, 0:4 * C], raw2[:, 4 * C:8 * C])
            nc.vector.tensor_copy(WA[:, 0:C], FU[:, 0:C])                 # S1
            nc.vector.tensor_copy(WA[:, C:2 * C], raw2[:, 4 * C:5 * C])   # S1r
            nc.vector.tensor_sub(WA[:, 2 * C:3 * C], xr[:, C:2 * C], xr[:, 0:C])
            nc.vector.tensor_copy(WB[:, 0:C], FU[:, C:2 * C])             # S2
            nc.vector.tensor_copy(WB[:, C:2 * C], raw2[:, 5 * C:6 * C])   # S2r
            nc.scalar.activation(out=SQ[:], in_=xr[:], func=ACT.Square)
            nc.vector.tensor_add(TMP[:, 0:C], SQ[:, 0:C], SQ[:, C:2 * C])
            nc.vector.tensor_sub(TMP[:, C:2 * C], FU[:, C:2 * C], FU[:, 3 * C:4 * C])
            nc.vector.scalar_tensor_tensor(                               # D2
                out=WB[:, 2 * C:3 * C], in0=TMP[:, C:2 * C], scalar=2.0,
                in1=TMP[:, 0:C], op0=ALU.mult, op1=ALU.subtract)

            nc.vector.tensor_mul(MU[:], WA[:], K1[:])                     # means
            nc.scalar.activation(out=SQ2[:], in_=MU[:], func=ACT.Square)
            nc.vector.tensor_mul(V[:], SQ2[:], KNEG[:])                   # -n*mu^2
            nc.vector.tensor_add(V[:], V[:], WB[:])                       # var numer
            nc.vector.tensor_mul(V[:], V[:], KINV[:])                     # var
            nc.scalar.activation(out=STD[:], in_=V[:], func=ACT.Sqrt)
            nc.vector.tensor_add(STD[:], STD[:], KEPS[:])

            # skew = (S3 - 3*mean*S2 + 2*N*mean^3) / (N * std^3)
            nc.vector.tensor_mul(TMP[:, 0:C], MU[:, 0:C], WB[:, 0:C])     # mean*S2
            nc.vector.tensor_mul(TMP[:, C:2 * C], SQ2[:, 0:C], MU[:, 0:C])
            nc.vector.scalar_tensor_tensor(
                out=TMP[:, C:2 * C], in0=TMP[:, C:2 * C], scalar=2.0 * N,
                in1=FU[:, 2 * C:3 * C], op0=ALU.mult, op1=ALU.add)
            nc.vector.scalar_tensor_tensor(
                out=TMP[:, 0:C], in0=TMP[:, 0:C], scalar=-3.0,
                in1=TMP[:, C:2 * C], op0=ALU.mult, op1=ALU.add)
            nc.vector.reciprocal(TMP[:, 2 * C:3 * C], STD[:, 0:C])
            nc.vector.tensor_mul(SKW[:], TMP[:, 2 * C:3 * C], TMP[:, 2 * C:3 * C])
            nc.vector.tensor_mul(SKW[:], SKW[:], TMP[:, 2 * C:3 * C])
            nc.vector.scalar_tensor_tensor(
                out=SKW[:], in0=TMP[:, 0:C], scalar=1.0 / N, in1=SKW[:],
                op0=ALU.mult, op1=ALU.mult)

            # ---------------- transpose stats to [128, 48] --------------------
            psT = pse.tile([128, 48], F32, tag="psT")
            blocks = [MU[:, 0:C], STD[:, 0:C], SKW[:], STD[:, 2 * C:3 * C],
                      MU[:, C:2 * C], STD[:, C:2 * C]]
            for v, blk in enumerate(blocks):
                # explicit start/stop: start=True would clear the whole bank
                nc.tensor.matmul(psT[:, 8 * v:8 * v + 8], blk, idsb[:],
                                 is_transpose=True, start=(v == 0),
                                 stop=(v == len(blocks) - 1),
                                 skip_group_check=True)
            statsT = fin.tile([128, 48], F32, tag="statsT")
            nc.vector.tensor_copy(statsT[:], psT[:])

            # ---------------- MLP (transposed: [feat, sample]) ----------------
            psH1 = pse.tile([128, 8], F32, tag="psH1")
            for k in range(6):
                nc.tensor.matmul(psH1[:], w1sb[:, k, :], statsT[:, 8 * k:8 * k + 8],
                                 start=(k == 0), stop=False)
            nc.tensor.matmul(psH1[:], b1sb[:], ones1[:], start=False, stop=True)

            esb = fin.tile([128, 8], F32, tag="esb")
            nc.scalar.activation(out=esb[:], in_=psH1[:], func=ACT.Erf,
                                 scale=float(1.0 / np.sqrt(2.0)))
            nc.vector.tensor_scalar(out=esb[:], in0=esb[:], scalar1=1.0, scalar2=0.5,
                                    op0=ALU.add, op1=ALU.mult)
            h1sb = fin.tile([128, 8], F32, tag="h1sb")
            nc.vector.tensor_mul(h1sb[:], esb[:], psH1[:])

            psH2 = pse.tile([32, 8], F32, tag="psH2")
            nc.tensor.matmul(psH2[:], w2sb[:], h1sb[:], start=True, stop=False)
            nc.tensor.matmul(psH2[:], b2sb[:], ones1[:], start=False, stop=True)
            esb2 = fin.tile([32, 8], F32, tag="esb2")
            nc.scalar.activation(out=esb2[:], in_=psH2[:], func=ACT.Erf,
                                 scale=float(1.0 / np.sqrt(2.0)))
            nc.vector.tensor_scalar(out=esb2[:], in0=esb2[:], scalar1=1.0, scalar2=0.5,
                                    op0=ALU.add, op1=ALU.mult)
            h2sb = fin.tile([32, 8], F32, tag="h2sb")
            nc.vector.tensor_mul(h2sb[:], esb2[:], psH2[:])

            psH3 = pse.tile([128, 8], F32, tag="psH3")
            nc.tensor.matmul(psH3[:], w3sb[:], h2sb[:], start=True, stop=False)
            nc.tensor.matmul(psH3[:], b3sb[:], ones1[:], start=False, stop=True)
            alphas = fin.tile([128, 8], F32, tag="alphas")
            nc.scalar.activation(out=alphas[:], in_=psH3[:], func=ACT.Sigmoid)

            nc.sync.dma_start(out=out[:], in_=alphas[:])
    nc.compile()
    return nc


_NC_CACHE = None


def _get_nc():
    global _NC_CACHE
    if _NC_CACHE is None:
        _NC_CACHE = build()
    return _NC_CACHE


def _run(inputs, **kwargs):
    x = np.ascontiguousarray(np.asarray(inputs["x"], dtype=np.float32))
    args = {k: np.ascontiguousarray(np.asarray(inputs[k], dtype=np.float32))
            for k in ("W1", "b1", "W2", "b2", "W3", "b3")}
    nc = _get_nc()
    in_maps = [dict(args, x=x[i * BS:(i + 1) * BS]) for i in range(NCORES)]
    res = run_bass_kernel_spmd(nc, in_maps, core_ids=list(range(NCORES)), **kwargs)
    out = np.concatenate([r["out"].T for r in res.results], axis=0)
    return out, res


def kernel(x, W1, b1, W2, b2, W3, b3):
    out, _ = _run(dict(x=x, W1=W1, b1=b1, W2=W2, b2=b2, W3=W3, b3=b3))
    return out

